# revision 21
# baseline (speedup 1.0000x reference)
"""Causal multi-head attention (B=1, H=16, S=2048, D=128, fp32 I/O) on 8 trn2 cores.

Sharding: 2 heads per core (batch*head data parallel). Each core runs the same
Bass/Tile program on its own head pair.

Device algorithm (per head), v2:
  - Host supplies Q^T, K^T as fp16 [128 d, 2048 s] and V packed as fp16
    [128 k, 16*129] (per k-tile: 128 V columns + a ones column).
  - Stage 1 (per k-tile row kt): S^T[kt] = K_kt^T.T @ Q^T -> PSUM fp32 over the
    causal column range [kt*128, 2048), in 1-2 chunks. No mask matmuls: the
    strictly-future entries of the diagonal block are exp'd like everything
    else and then zeroed in SBUF by a GpSimd affine_select (GpSimd is idle).
  - exp is SPLIT between ScalarE (exact table exp, PSUM->SBUF fp16) and
    VectorE (Schraudolph fast exp: one tensor_scalar computing
    i16 = int(x*A + B) whose bit pattern IS fp16 exp(x*scale); |rel err| ~3%
    per element, ~0.4% on the softmax output because the denominator is
    computed from the same approximated values). Chunks are assigned to the
    two engines greedily by modeled cumulative cost, so both stay busy and
    the exp stream runs ~1.8x faster than ScalarE alone.
  - Stage 2 (per q-tile qt): accumulate over kt <= qt:
    acc[128 q, 129] += P^T[kt][:, qt-block].T @ V_aug[kt]
    -> columns 0..127 are the UNNORMALIZED output, column 128 the softmax
    denominator. Three consecutive q-tiles share one PSUM bank
    ([128, 3*129] <= 512 fp32), so one engine copy ships three tiles at once.
  - NO on-device normalization: the acc triples are copied PSUM->SBUF fp16
    (ScalarE/VectorE, balance-scheduled) and DMA'd out unnormalized; the host
    divides by the denominator column. This frees VectorE for exp work.

Startup: the two DMAs that gate the first matmul (K row 0, Q head) go on the
VECTOR DGE ring, whose framework preamble finishes ~1.5us before sync's.
Warm-up matmuls on a zeroed tile run during the DMA wait so the PE HAM clock
gate reaches 2.4 GHz before the real matmuls; the ScalarE exp table is
preloaded with a dummy activation at the same time.
"""

import os
import sys

import numpy as np

if "/opt/trn_rl_repo" not in sys.path:
    sys.path.insert(0, "/opt/trn_rl_repo")

B, H, S, D = 1, 16, 2048, 128
N_CORES = 8
HPC = H // N_CORES  # heads per core
NT = S // 128  # 16 seq tiles
VW = D + 1  # 129: V columns + ones column
SCALE = 1.0 / float(np.sqrt(D))
CHUNK = 1536  # stage-1 PSUM chunk (3 banks, 2 bufs -> depth-2 pipeline)

# Schraudolph fast-exp constants (fp16 bit space):
#   i16 = int(s_raw * EXP_A + EXP_B);  bitcast fp16 ~= exp(s_raw * SCALE)
# EXP_A folds the softmax scale; EXP_B = 15*1024 - c with c~60 minimizing the
# end-to-end softmax error (numpy-calibrated; insensitive to round-vs-trunc).
EXP_A = float(SCALE * 1024.0 * np.log2(np.e))
EXP_B = float(15 * 1024.0 - 60.0)

# Modeled per-instruction engine costs (ns) for the greedy exp/copy balance.
ACT_NS, ACT_FIX = 1.0 / 1.2, 257.0
DVE_NS, DVE_FIX = 1.0 / 0.96, 175.0

_CACHE = {}


def _build_program():
    if "nc" in _CACHE:
        return _CACHE["nc"]

    import concourse.bass as bass
    import concourse.mybir as mybir
    import concourse.tile as tile
    from concourse import bacc
    from contextlib import ExitStack

    f16 = mybir.dt.float16
    i16 = mybir.dt.int16
    f32 = mybir.dt.float32

    nc = bacc.Bacc("TRN2", target_bir_lowering=False, debug=False,
                   num_devices=N_CORES)

    qT = nc.dram_tensor("qT", [HPC, 128, S], f16, kind="ExternalInput").ap()
    kT = nc.dram_tensor("kT", [HPC, 128, S], f16, kind="ExternalInput").ap()
    vA = nc.dram_tensor("vA", [HPC, 128, NT * VW], f16, kind="ExternalInput").ap()
    # Output stays q-tile-partition-major ([q-offset, qt*VW+col]) so every
    # output DMA is per-partition contiguous; the host untangles it.
    out = nc.dram_tensor("out", [HPC, 128, NT * VW], f16,
                         kind="ExternalOutput").ap()

    with tile.TileContext(nc, pool_alloc_mode="queue") as tc, ExitStack() as ctx:
        const_pool = ctx.enter_context(tc.tile_pool(name="const", bufs=1))
        in_pool = ctx.enter_context(tc.tile_pool(name="qkv", bufs=2))
        # 2*NT bufs: every P^T row tile of both heads gets its own buffer, so
        # head 1's stage-1 never WAR-waits on head 0's stage-2 readers.
        p_pool = ctx.enter_context(tc.tile_pool(name="pT", bufs=2 * NT))
        o_pool = ctx.enter_context(tc.tile_pool(name="osb", bufs=4))
        s_psum = ctx.enter_context(tc.tile_pool(name="spsum", bufs=2, space="PSUM"))
        a_psum = ctx.enter_context(tc.tile_pool(name="apsum", bufs=2, space="PSUM"))

        # PE warm-up: the HAM clock gate keeps TensorE at 1.2 GHz until it has
        # been busy ~3.4us. Run throwaway matmuls on a zeroed tile while the
        # first input DMAs are in flight; the real matmuls then extend the
        # busy streak so HAM reaches 2.4 GHz ~3.4us after the first warm-up.
        # The memset rides VectorE (idle, fast) so the warm-ups start the
        # moment the framework preamble barrier drops.
        warm_sb = const_pool.tile([128, 512], f16)
        nc.vector.memset(warm_sb[:], 0.0)
        warm_ps = s_psum.tile([128, CHUNK], mybir.dt.float32, tag="s",
                              name="warm_ps")
        for _ in range(6):
            nc.tensor.matmul(warm_ps[:, 0:512], warm_sb[:, 0:128],
                             warm_sb[:, 0:512], start=True, stop=True)
        # Preload the ScalarE exp table set during the DMA wait (walrus puts
        # the ACT_TABLE_LOAD right before this first ACTIVATE).
        warm_exp = const_pool.tile([128, 1], f16)
        nc.scalar.activation(warm_exp[:], warm_sb[:, 0:1],
                             mybir.ActivationFunctionType.Exp, scale=SCALE)

        qk_sb = {}   # h -> (qT_sb, kT_sb, vA_sb)
        pT = {}      # h -> list of P^T row tiles

        def emit_loads(h, first=False):
            qT_sb = in_pool.tile([128, S], f16, tag="q", name=f"q_{h}")
            kT_sb = in_pool.tile([128, S], f16, tag="k", name=f"k_{h}")
            vA_sb = in_pool.tile([128, NT * VW], f16, tag="v", name=f"v_{h}")
            if first:
                # Rows 12-15 run FIRST (see rows_order): they need only the
                # tail 640 columns of K and Q (320 KB), which land ~2us
                # before the full Q does at the ~180 GB/s per-queue DMA rate.
                # Those rows bridge the PE from the warm-up matmuls to row 0
                # with no idle gap, so the HAM clock gate reaches 2.4 GHz
                # before the bulk of the work. Q's head is split across the
                # sync and gpsimd rings so both queues carry it in parallel.
                nc.sync.dma_start(kT_sb[:, 1408:2048], kT[h][:, 1408:2048])
                nc.sync.dma_start(qT_sb[:, 1408:2048], qT[h][:, 1408:2048])
                nc.sync.dma_start(qT_sb[:, 0:704], qT[h][:, 0:704])
                nc.sync.dma_start(kT_sb[:, 0:128], kT[h][:, 0:128])
                nc.sync.dma_start(kT_sb[:, 128:640], kT[h][:, 128:640])
                nc.sync.dma_start(vA_sb[:, 0:6 * VW], vA[h][:, 0:6 * VW])
                nc.gpsimd.dma_start(qT_sb[:, 704:1408], qT[h][:, 704:1408])
                nc.gpsimd.dma_start(kT_sb[:, 640:1408], kT[h][:, 640:1408])
                nc.gpsimd.dma_start(vA_sb[:, 6 * VW:], vA[h][:, 6 * VW:])
            else:
                # Later heads run in natural row order: K row 0 + full Q
                # first, K tail and V after (first-use order).
                nc.sync.dma_start(kT_sb[:, 0:128], kT[h][:, 0:128])
                nc.sync.dma_start(qT_sb[:, 0:2048], qT[h][:, 0:2048])
                nc.sync.dma_start(kT_sb[:, 128:2048], kT[h][:, 128:2048])
                nc.sync.dma_start(vA_sb[:], vA[h])
            qk_sb[h] = (qT_sb, kT_sb, vA_sb)
            pT[h] = [p_pool.tile([128, S], f16, tag="p", name=f"p_{h}_{kt}")
                     for kt in range(NT)]

        # Greedy ACT/DVE balance for exp chunks and acc copies.
        eng_t = {"act": 0.0, "dve": 0.0}

        def pick_engine():
            return "act" if eng_t["act"] <= eng_t["dve"] else "dve"

        def emit_exp(engine, h, kt, lo, hi, sp, sp_lo):
            # exp of score chunk cols [lo, hi) of row kt (global q coords),
            # reading PSUM tile sp at offset lo - sp_lo.
            dst = pT[h][kt][:, lo:hi]
            src = sp[:, lo - sp_lo:hi - sp_lo]
            n = hi - lo
            if engine == "act":
                nc.scalar.activation(dst, src,
                                     mybir.ActivationFunctionType.Exp,
                                     scale=SCALE)
                eng_t["act"] += n * ACT_NS + ACT_FIX
            else:
                nc.vector.tensor_scalar(
                    dst.bitcast(i16), src, EXP_A, EXP_B,
                    mybir.AluOpType.mult, mybir.AluOpType.add)
                eng_t["dve"] += n * DVE_NS + DVE_FIX

        def stage1(h, kt, splits=None):
            qT_sb, kT_sb, _ = qk_sb[h]
            c0 = kt * 128
            k_blk = kT_sb[:, c0:c0 + 128]
            L = S - c0
            if splits is None:
                splits = [CHUNK, L - CHUNK] if L > CHUNK else [L]
            cc = c0
            first = True
            for clen in splits:
                sp = s_psum.tile([128, CHUNK], mybir.dt.float32, tag="s",
                                 name=f"sp_{h}_{kt}_{cc}")
                mo = 0
                while mo < clen:
                    # Matmul outputs must stay within one PSUM bank (512 fp32).
                    mlen = min(512, clen - mo)
                    nc.tensor.matmul(
                        sp[:, mo:mo + mlen],
                        k_blk,
                        qT_sb[:, cc + mo:cc + mo + mlen],
                        start=True, stop=True,
                    )
                    mo += mlen
                # Chunks >= 768 are split across BOTH engines so the PSUM
                # tile frees ~2x sooner: with 2 s_psum bufs the PE's chunk
                # i+2 matmuls WAR-wait on chunk i's exp, and a single-engine
                # 1536-col exp (~1.5us) exceeds the PE work in between.
                # Small chunks go whole to whichever engine is behind.
                if clen >= 768:
                    if eng_t["act"] <= eng_t["dve"]:
                        a = int((1.042 * clen - 82) / 1.875)
                        a = max(128, min(clen - 64, a // 2 * 2))
                        emit_exp("act", h, kt, cc, cc + a, sp, cc)
                        emit_exp("dve", h, kt, cc + a, cc + clen, sp, cc)
                    else:
                        a = int((0.833 * clen + 82) / 1.875)
                        a = max(128, min(clen - 64, a // 2 * 2))
                        emit_exp("dve", h, kt, cc, cc + a, sp, cc)
                        emit_exp("act", h, kt, cc + a, cc + clen, sp, cc)
                else:
                    emit_exp(pick_engine(), h, kt, cc, cc + clen, sp, cc)
                if first:
                    # Zero the strictly-future entries of the diagonal block
                    # (k > q <=> partition p > col j) now that exp ran. The
                    # subsequent PV matmuls and the ones-column denominator
                    # then see exact causal zeros. GpSimd is otherwise idle.
                    diag = pT[h][kt][:, c0:c0 + 128]
                    nc.gpsimd.affine_select(
                        diag, diag, pattern=[[1, 128]],
                        compare_op=mybir.AluOpType.is_ge, fill=0.0,
                        base=0, channel_multiplier=-1)
                cc += clen
                first = False

        accs = {}

        def ship_triple(h, trip, nq):
            # Copy the finished acc triple PSUM->SBUF fp16 on the engine
            # that's ahead, then DMA it out unnormalized (host divides).
            acc = accs[(h, trip)]
            w = nq * VW
            osb = o_pool.tile([128, w], f16, tag="o", name=f"osb_{h}_{trip}")
            eng = pick_engine()
            if eng == "act":
                nc.scalar.copy(osb[:], acc[:, :w])
                eng_t["act"] += w * ACT_NS + ACT_FIX
            else:
                nc.vector.tensor_copy(osb[:], acc[:, :w])
                eng_t["dve"] += w * DVE_NS + DVE_FIX
            dst = out[h][:, trip * 3 * VW:(trip * 3 + nq) * VW]
            # Output DMAs ride the idle gpsimd ring; the tail triples of the
            # last head go on sync, which is free once inputs are done.
            q = nc.sync if (h == HPC - 1 and trip >= 4) else nc.gpsimd
            q.dma_start(dst, osb[:])

        def stage2_piece(h, qt, lo, hi):
            # One slice of the PV accumulation group for q-tile qt. PSUM
            # accumulation is per-element, so the group's matmuls need not be
            # contiguous on the PE stream.
            vA_sb = qk_sb[h][2]
            q0 = qt * 128
            trip, slot = qt // 3, qt % 3
            if lo == 0 and slot == 0:
                accs[(h, trip)] = a_psum.tile([128, 3 * VW], mybir.dt.float32,
                                              tag="acc", name=f"acc_{h}_{trip}")
            acc = accs[(h, trip)][:, slot * VW:(slot + 1) * VW]
            for k2 in range(lo, hi):
                nc.tensor.matmul(
                    acc,
                    pT[h][k2][:, q0:q0 + 128],
                    vA_sb[:, k2 * VW:(k2 + 1) * VW],
                    start=(k2 == 0), stop=(k2 == qt),
                )
            if hi == qt + 1 and (slot == 2 or qt == NT - 1):
                ship_triple(h, trip, slot + 1)

        # One flat software pipeline across both heads. Each head's rows run
        # in the order [12..15, 0..11]: the tiny tail rows bridge the PE over
        # the Q-head DMA window (they need only late K/Q columns). Stage-2
        # pieces are slotted by data-readiness: a piece runs 1-2 slots after
        # the LAST stage-1 row it depends on. Groups 10-15 therefore drain at
        # the head boundary, naturally interleaving with the next head's
        # early rows (PE chews PV bulks while the exp engines chew the next
        # head's scores).
        # Head 0 needs the bridge reorder; later heads' inputs land while
        # head 0 computes, so they run in natural order — their PV groups
        # then spread across their own rows and the final drain is short.
        PRE = [12, 13, 14, 15]
        seq = []
        for h in range(HPC):
            order = PRE + list(range(12)) if h == 0 else list(range(NT))
            seq += [(h, kt) for kt in order]
        gpos = {hk: i for i, hk in enumerate(seq)}
        NSLOT = len(seq) + 6
        pieces = [[] for _ in range(NSLOT)]

        # Every group is ONE whole piece, slotted 2 after its last required
        # row. Splitting groups into early-bulk pieces looks tempting but
        # overlaps three acc-triple lifetimes, which WAR-deadlocks the
        # in-order PE against the 2-buf a_psum pool. Whole groups keep the
        # triple windows strictly 2-deep: triple t ships at slot 3t+8,
        # before triple t+2 allocates at slot 3t+12.
        for h in range(HPC):
            for qt in range(NT):
                ready = max(gpos[(h, r)] for r in range(qt + 1))
                pieces[min(NSLOT - 1, ready + 2)].append((h, qt, 0, qt + 1))

        emit_loads(0, first=True)
        started = {0}
        for i, (h, kt) in enumerate(seq):
            if h + 1 < HPC and kt == 15 and (h + 1) not in started:
                emit_loads(h + 1)
                started.add(h + 1)
            stage1(h, kt)
            for p in pieces[i]:
                stage2_piece(*p)
        for pl in pieces[len(seq):]:
            for p in pl:
                stage2_piece(*p)

    nc.compile()
    _CACHE["nc"] = nc
    return nc


def _host_prep(query_states, key_states, value_states):
    """Per-core input maps: fp16 Q^T/K^T and ones-augmented V."""
    q = np.asarray(query_states, dtype=np.float32).reshape(H, S, D)
    k = np.asarray(key_states, dtype=np.float32).reshape(H, S, D)
    v = np.asarray(value_states, dtype=np.float32).reshape(H, S, D)

    in_maps = []
    for c in range(N_CORES):
        hs = slice(c * HPC, (c + 1) * HPC)
        qT = np.ascontiguousarray(
            q[hs].transpose(0, 2, 1).astype(np.float16))  # [HPC,128,S]
        kT = np.ascontiguousarray(
            k[hs].transpose(0, 2, 1).astype(np.float16))
        vh = v[hs].astype(np.float16).reshape(HPC, NT, 128, D)
        vA = np.empty((HPC, 128, NT * VW), dtype=np.float16)
        for hh in range(HPC):
            for kt in range(NT):
                vA[hh, :, kt * VW:kt * VW + D] = vh[hh, kt]
                vA[hh, :, kt * VW + D] = np.float16(1.0)
        in_maps.append({"qT": qT, "kT": kT, "vA": vA})
    return in_maps


def run_cores(in_maps, trace=False, **kw):
    from concourse.bass_utils import run_bass_kernel_spmd
    nc = _build_program()
    return run_bass_kernel_spmd(nc, in_maps, list(range(N_CORES)),
                                trace=trace, **kw)


def kernel(query_states, key_states, value_states, attention_mask=None,
           attention_dropout=None, **_ignored):
    in_maps = _host_prep(query_states, key_states, value_states)
    res = run_cores(in_maps)
    outs = []
    for c in range(N_CORES):
        o = np.asarray(res.results[c]["out"], dtype=np.float32)  # [HPC,128,NT*VW]
        o = o.reshape(HPC, 128, NT, VW).transpose(0, 2, 1, 3)  # [HPC,NT,128,VW]
        o = o[..., :D] / o[..., D:D + 1]  # host-side softmax normalization
        outs.append(o.reshape(HPC, S, D))
    full = np.concatenate(outs, axis=0).reshape(B, H, S, D).astype(np.float32)
    return full


# revision 26
# speedup vs baseline: 1.0493x; 1.0493x over previous
"""Causal multi-head attention (B=1, H=16, S=2048, D=128, fp32 I/O) on 8 trn2 cores.

Sharding: 2 heads per core (batch*head data parallel). Each core runs the same
Bass/Tile program on its own head pair.

Device algorithm (per head), v2:
  - Host supplies Q^T, K^T as fp16 [128 d, 2048 s] and V packed as fp16
    [128 k, 16*129] (per k-tile: 128 V columns + a ones column).
  - Stage 1 (per k-tile row kt): S^T[kt] = K_kt^T.T @ Q^T -> PSUM fp32 over the
    causal column range [kt*128, 2048), in 1-2 chunks. No mask matmuls: the
    strictly-future entries of the diagonal block are exp'd like everything
    else and then zeroed in SBUF by a GpSimd affine_select (GpSimd is idle).
  - exp is SPLIT between ScalarE (exact table exp, PSUM->SBUF fp16) and
    VectorE (Schraudolph fast exp: one tensor_scalar computing
    i16 = int(x*A + B) whose bit pattern IS fp16 exp(x*scale); |rel err| ~3%
    per element, ~0.4% on the softmax output because the denominator is
    computed from the same approximated values). Chunks are assigned to the
    two engines greedily by modeled cumulative cost, so both stay busy and
    the exp stream runs ~1.8x faster than ScalarE alone.
  - Stage 2 (per q-tile qt): accumulate over kt <= qt:
    acc[128 q, 129] += P^T[kt][:, qt-block].T @ V_aug[kt]
    -> columns 0..127 are the UNNORMALIZED output, column 128 the softmax
    denominator. Three consecutive q-tiles share one PSUM bank
    ([128, 3*129] <= 512 fp32), so one engine copy ships three tiles at once.
  - NO on-device normalization: the acc triples are copied PSUM->SBUF fp16
    (ScalarE/VectorE, balance-scheduled) and DMA'd out unnormalized; the host
    divides by the denominator column. This frees VectorE for exp work.

Startup: the two DMAs that gate the first matmul (K row 0, Q head) go on the
VECTOR DGE ring, whose framework preamble finishes ~1.5us before sync's.
Warm-up matmuls on a zeroed tile run during the DMA wait so the PE HAM clock
gate reaches 2.4 GHz before the real matmuls; the ScalarE exp table is
preloaded with a dummy activation at the same time.
"""

import os
import sys

import numpy as np

if "/opt/trn_rl_repo" not in sys.path:
    sys.path.insert(0, "/opt/trn_rl_repo")

B, H, S, D = 1, 16, 2048, 128
N_CORES = 8
HPC = H // N_CORES  # heads per core
NT = S // 128  # 16 seq tiles
VW = D + 1  # 129: V columns + ones column
SCALE = 1.0 / float(np.sqrt(D))
CHUNK = 1536  # stage-1 PSUM chunk (3 banks, 2 bufs -> depth-2 pipeline)

# Schraudolph fast-exp constants (fp16 bit space):
#   i16 = int(s_raw * EXP_A + EXP_B);  bitcast fp16 ~= exp(s_raw * SCALE)
# EXP_A folds the softmax scale; EXP_B = 15*1024 - c with c~60 minimizing the
# end-to-end softmax error (numpy-calibrated; insensitive to round-vs-trunc).
EXP_A = float(SCALE * 1024.0 * np.log2(np.e))
EXP_B = float(15 * 1024.0 - 60.0)

# Modeled per-instruction engine costs (ns) for the greedy exp/copy balance.
ACT_NS, ACT_FIX = 1.0 / 1.2, 257.0
DVE_NS, DVE_FIX = 1.0 / 0.96, 175.0

_CACHE = {}


def _build_program():
    if "nc" in _CACHE:
        return _CACHE["nc"]

    import concourse.bass as bass
    import concourse.mybir as mybir
    import concourse.tile as tile
    from concourse import bacc
    from contextlib import ExitStack

    f16 = mybir.dt.float16
    i16 = mybir.dt.int16
    f32 = mybir.dt.float32

    nc = bacc.Bacc("TRN2", target_bir_lowering=False, debug=False,
                   num_devices=N_CORES)

    qT = nc.dram_tensor("qT", [HPC, 128, S], f16, kind="ExternalInput").ap()
    kT = nc.dram_tensor("kT", [HPC, 128, S], f16, kind="ExternalInput").ap()
    vA = nc.dram_tensor("vA", [HPC, 128, NT * VW], f16, kind="ExternalInput").ap()
    # Output stays q-tile-partition-major ([q-offset, qt*VW+col]) so every
    # output DMA is per-partition contiguous; the host untangles it.
    out = nc.dram_tensor("out", [HPC, 128, NT * VW], f16,
                         kind="ExternalOutput").ap()

    with tile.TileContext(nc, pool_alloc_mode="queue") as tc, ExitStack() as ctx:
        const_pool = ctx.enter_context(tc.tile_pool(name="const", bufs=1))
        in_pool = ctx.enter_context(tc.tile_pool(name="qkv", bufs=2))
        # One buffer per exp-piece tile across both heads (~80 pieces), so
        # no stage-1 writer ever WAR-waits on a stage-2 reader.
        p_pool = ctx.enter_context(tc.tile_pool(name="pT", bufs=96))
        o_pool = ctx.enter_context(tc.tile_pool(name="osb", bufs=4))
        s_psum = ctx.enter_context(tc.tile_pool(name="spsum", bufs=2, space="PSUM"))
        a_psum = ctx.enter_context(tc.tile_pool(name="apsum", bufs=2, space="PSUM"))

        # PE warm-up: the HAM clock gate keeps TensorE at 1.2 GHz until it has
        # been busy ~3.4us. Run throwaway matmuls on a zeroed tile while the
        # first input DMAs are in flight; the real matmuls then extend the
        # busy streak so HAM reaches 2.4 GHz ~3.4us after the first warm-up.
        # The memset rides VectorE (idle, fast) so the warm-ups start the
        # moment the framework preamble barrier drops.
        warm_sb = const_pool.tile([128, 512], f16)
        nc.vector.memset(warm_sb[:], 0.0)
        warm_ps = s_psum.tile([128, CHUNK], mybir.dt.float32, tag="s",
                              name="warm_ps")
        for _ in range(6):
            nc.tensor.matmul(warm_ps[:, 0:512], warm_sb[:, 0:128],
                             warm_sb[:, 0:512], start=True, stop=True)
        # Preload the ScalarE exp table set during the DMA wait (walrus puts
        # the ACT_TABLE_LOAD right before this first ACTIVATE).
        warm_exp = const_pool.tile([128, 1], f16)
        nc.scalar.activation(warm_exp[:], warm_sb[:, 0:1],
                             mybir.ActivationFunctionType.Exp, scale=SCALE)

        qk_sb = {}   # h -> (qT_sb, kT_sb, vA_sb)
        pT = {}      # h -> list of P^T row tiles

        def emit_loads(h, first=False):
            qT_sb = in_pool.tile([128, S], f16, tag="q", name=f"q_{h}")
            kT_sb = in_pool.tile([128, S], f16, tag="k", name=f"k_{h}")
            vA_sb = in_pool.tile([128, NT * VW], f16, tag="v", name=f"v_{h}")
            if first:
                # Rows 12-15 run FIRST (see rows_order): they need only the
                # tail 640 columns of K and Q (320 KB), which land ~2us
                # before the full Q does at the ~180 GB/s per-queue DMA rate.
                # Those rows bridge the PE from the warm-up matmuls to row 0
                # with no idle gap, so the HAM clock gate reaches 2.4 GHz
                # before the bulk of the work. Q's head is split across the
                # sync and gpsimd rings so both queues carry it in parallel.
                nc.sync.dma_start(kT_sb[:, 1408:2048], kT[h][:, 1408:2048])
                nc.sync.dma_start(qT_sb[:, 1408:2048], qT[h][:, 1408:2048])
                nc.sync.dma_start(qT_sb[:, 0:704], qT[h][:, 0:704])
                nc.sync.dma_start(kT_sb[:, 0:128], kT[h][:, 0:128])
                nc.sync.dma_start(kT_sb[:, 128:640], kT[h][:, 128:640])
                nc.sync.dma_start(vA_sb[:, 0:6 * VW], vA[h][:, 0:6 * VW])
                nc.gpsimd.dma_start(qT_sb[:, 704:1408], qT[h][:, 704:1408])
                nc.gpsimd.dma_start(kT_sb[:, 640:1408], kT[h][:, 640:1408])
                nc.gpsimd.dma_start(vA_sb[:, 6 * VW:], vA[h][:, 6 * VW:])
            else:
                # Later heads run in natural row order: K row 0 + full Q
                # first, K tail and V after (first-use order).
                nc.sync.dma_start(kT_sb[:, 0:128], kT[h][:, 0:128])
                nc.sync.dma_start(qT_sb[:, 0:2048], qT[h][:, 0:2048])
                nc.sync.dma_start(kT_sb[:, 128:2048], kT[h][:, 128:2048])
                nc.sync.dma_start(vA_sb[:], vA[h])
            qk_sb[h] = (qT_sb, kT_sb, vA_sb)
            pT[h] = {kt: [] for kt in range(NT)}  # kt -> [(g_lo, g_hi, tile)]

        # Greedy ACT/DVE balance for exp chunks and acc copies.
        eng_t = {"act": 0.0, "dve": 0.0}

        def pick_engine():
            return "act" if eng_t["act"] <= eng_t["dve"] else "dve"

        def emit_exp(engine, h, kt, lo, hi, sp, sp_lo):
            # exp of score chunk cols [lo, hi) of row kt (global q coords),
            # reading PSUM tile sp at offset lo - sp_lo. Each piece gets its
            # OWN SBUF tile: the Tile framework serializes two engines that
            # write disjoint ranges of one tile (cross-engine same-tile
            # hazard), which would turn the ACT||DVE split back into a
            # serial stream. One writer per tile keeps them parallel.
            n = hi - lo
            t = p_pool.tile([128, n], f16, tag="p", name=f"p_{h}_{kt}_{lo}")
            pT[h][kt].append((lo, hi, t))
            src = sp[:, lo - sp_lo:hi - sp_lo]
            if engine == "act":
                nc.scalar.activation(t[:], src,
                                     mybir.ActivationFunctionType.Exp,
                                     scale=SCALE)
                eng_t["act"] += n * ACT_NS + ACT_FIX
            else:
                nc.vector.tensor_scalar(
                    t[:].bitcast(i16), src, EXP_A, EXP_B,
                    mybir.AluOpType.mult, mybir.AluOpType.add)
                eng_t["dve"] += n * DVE_NS + DVE_FIX

        def stage1(h, kt, splits=None):
            qT_sb, kT_sb, _ = qk_sb[h]
            c0 = kt * 128
            k_blk = kT_sb[:, c0:c0 + 128]
            L = S - c0
            if splits is None:
                splits = [CHUNK, L - CHUNK] if L > CHUNK else [L]
            cc = c0
            first = True
            for clen in splits:
                sp = s_psum.tile([128, CHUNK], mybir.dt.float32, tag="s",
                                 name=f"sp_{h}_{kt}_{cc}")
                mo = 0
                while mo < clen:
                    # Matmul outputs must stay within one PSUM bank (512 fp32).
                    mlen = min(512, clen - mo)
                    nc.tensor.matmul(
                        sp[:, mo:mo + mlen],
                        k_blk,
                        qT_sb[:, cc + mo:cc + mo + mlen],
                        start=True, stop=True,
                    )
                    mo += mlen
                # Chunks >= 768 are split across BOTH engines: the two
                # halves exp in parallel, so the PSUM tile frees ~2x sooner
                # (with 2 s_psum bufs the PE's chunk i+2 matmuls WAR-wait on
                # chunk i's exp). Split points are 128-aligned so every
                # stage-2 q-block lies inside exactly one piece tile.
                # Small chunks go whole to whichever engine is behind.
                if clen >= 768:
                    if eng_t["act"] <= eng_t["dve"]:
                        a = int((1.042 * clen - 82) / 1.875 / 128 + 0.5) * 128
                        a = max(128, min(clen - 128, a))
                        emit_exp("act", h, kt, cc, cc + a, sp, cc)
                        emit_exp("dve", h, kt, cc + a, cc + clen, sp, cc)
                    else:
                        a = int((0.833 * clen + 82) / 1.875 / 128 + 0.5) * 128
                        a = max(128, min(clen - 128, a))
                        emit_exp("dve", h, kt, cc, cc + a, sp, cc)
                        emit_exp("act", h, kt, cc + a, cc + clen, sp, cc)
                else:
                    emit_exp(pick_engine(), h, kt, cc, cc + clen, sp, cc)
                if first:
                    # Zero the strictly-future entries of the diagonal block
                    # (k > q <=> partition p > col j) now that exp ran. The
                    # subsequent PV matmuls and the ones-column denominator
                    # then see exact causal zeros. GpSimd is otherwise idle.
                    diag = pT[h][kt][0][2][:, 0:128]
                    nc.gpsimd.affine_select(
                        diag, diag, pattern=[[1, 128]],
                        compare_op=mybir.AluOpType.is_ge, fill=0.0,
                        base=0, channel_multiplier=-1)
                cc += clen
                first = False

        accs = {}

        def ship_triple(h, trip, nq):
            # Copy the finished acc triple PSUM->SBUF fp16 on the engine
            # that's ahead, then DMA it out unnormalized (host divides).
            acc = accs[(h, trip)]
            w = nq * VW
            osb = o_pool.tile([128, w], f16, tag="o", name=f"osb_{h}_{trip}")
            eng = pick_engine()
            if eng == "act":
                nc.scalar.copy(osb[:], acc[:, :w])
                eng_t["act"] += w * ACT_NS + ACT_FIX
            else:
                nc.vector.tensor_copy(osb[:], acc[:, :w])
                eng_t["dve"] += w * DVE_NS + DVE_FIX
            dst = out[h][:, trip * 3 * VW:(trip * 3 + nq) * VW]
            # Output DMAs ride the idle gpsimd ring; the tail triples of the
            # last head go on sync, which is free once inputs are done.
            q = nc.sync if (h == HPC - 1 and trip >= 4) else nc.gpsimd
            q.dma_start(dst, osb[:])

        def stage2_piece(h, qt, lo, hi):
            # One slice of the PV accumulation group for q-tile qt. PSUM
            # accumulation is per-element, so the group's matmuls need not be
            # contiguous on the PE stream.
            vA_sb = qk_sb[h][2]
            q0 = qt * 128
            trip, slot = qt // 3, qt % 3
            if lo == 0 and slot == 0:
                accs[(h, trip)] = a_psum.tile([128, 3 * VW], mybir.dt.float32,
                                              tag="acc", name=f"acc_{h}_{trip}")
            acc = accs[(h, trip)][:, slot * VW:(slot + 1) * VW]
            for k2 in range(lo, hi):
                for (g_lo, g_hi, t) in pT[h][k2]:
                    if g_lo <= q0 < g_hi:
                        blk = t[:, q0 - g_lo:q0 - g_lo + 128]
                        break
                else:
                    raise AssertionError((h, k2, q0))
                nc.tensor.matmul(
                    acc,
                    blk,
                    vA_sb[:, k2 * VW:(k2 + 1) * VW],
                    start=(k2 == 0), stop=(k2 == qt),
                )
            if hi == qt + 1 and (slot == 2 or qt == NT - 1):
                ship_triple(h, trip, slot + 1)

        # One flat software pipeline across both heads. Each head's rows run
        # in the order [12..15, 0..11]: the tiny tail rows bridge the PE over
        # the Q-head DMA window (they need only late K/Q columns). Stage-2
        # pieces are slotted by data-readiness: a piece runs 1-2 slots after
        # the LAST stage-1 row it depends on. Groups 10-15 therefore drain at
        # the head boundary, naturally interleaving with the next head's
        # early rows (PE chews PV bulks while the exp engines chew the next
        # head's scores).
        # Head 0 needs the bridge reorder; later heads' inputs land while
        # head 0 computes, so they run in natural order — their PV groups
        # then spread across their own rows and the final drain is short.
        PRE = [12, 13, 14, 15]
        seq = []
        for h in range(HPC):
            order = PRE + list(range(12)) if h == 0 else list(range(NT))
            seq += [(h, kt) for kt in order]
        gpos = {hk: i for i, hk in enumerate(seq)}
        NSLOT = len(seq) + 6
        pieces = [[] for _ in range(NSLOT)]

        # Every group is ONE whole piece, slotted 2 after its last required
        # row. Splitting groups into early-bulk pieces looks tempting but
        # overlaps three acc-triple lifetimes, which WAR-deadlocks the
        # in-order PE against the 2-buf a_psum pool. Whole groups keep the
        # triple windows strictly 2-deep: triple t ships at slot 3t+8,
        # before triple t+2 allocates at slot 3t+12.
        for h in range(HPC):
            for qt in range(NT):
                ready = max(gpos[(h, r)] for r in range(qt + 1))
                pieces[min(NSLOT - 1, ready + 2)].append((h, qt, 0, qt + 1))

        emit_loads(0, first=True)
        started = {0}
        for i, (h, kt) in enumerate(seq):
            if h + 1 < HPC and kt == 15 and (h + 1) not in started:
                emit_loads(h + 1)
                started.add(h + 1)
            stage1(h, kt)
            for p in pieces[i]:
                stage2_piece(*p)
        for pl in pieces[len(seq):]:
            for p in pl:
                stage2_piece(*p)

    nc.compile()
    _CACHE["nc"] = nc
    return nc


def _host_prep(query_states, key_states, value_states):
    """Per-core input maps: fp16 Q^T/K^T and ones-augmented V."""
    q = np.asarray(query_states, dtype=np.float32).reshape(H, S, D)
    k = np.asarray(key_states, dtype=np.float32).reshape(H, S, D)
    v = np.asarray(value_states, dtype=np.float32).reshape(H, S, D)

    in_maps = []
    for c in range(N_CORES):
        hs = slice(c * HPC, (c + 1) * HPC)
        qT = np.ascontiguousarray(
            q[hs].transpose(0, 2, 1).astype(np.float16))  # [HPC,128,S]
        kT = np.ascontiguousarray(
            k[hs].transpose(0, 2, 1).astype(np.float16))
        vh = v[hs].astype(np.float16).reshape(HPC, NT, 128, D)
        vA = np.empty((HPC, 128, NT * VW), dtype=np.float16)
        for hh in range(HPC):
            for kt in range(NT):
                vA[hh, :, kt * VW:kt * VW + D] = vh[hh, kt]
                vA[hh, :, kt * VW + D] = np.float16(1.0)
        in_maps.append({"qT": qT, "kT": kT, "vA": vA})
    return in_maps


def run_cores(in_maps, trace=False, **kw):
    from concourse.bass_utils import run_bass_kernel_spmd
    nc = _build_program()
    return run_bass_kernel_spmd(nc, in_maps, list(range(N_CORES)),
                                trace=trace, **kw)


def kernel(query_states, key_states, value_states, attention_mask=None,
           attention_dropout=None, **_ignored):
    in_maps = _host_prep(query_states, key_states, value_states)
    res = run_cores(in_maps)
    outs = []
    for c in range(N_CORES):
        o = np.asarray(res.results[c]["out"], dtype=np.float32)  # [HPC,128,NT*VW]
        o = o.reshape(HPC, 128, NT, VW).transpose(0, 2, 1, 3)  # [HPC,NT,128,VW]
        o = o[..., :D] / o[..., D:D + 1]  # host-side softmax normalization
        outs.append(o.reshape(HPC, S, D))
    full = np.concatenate(outs, axis=0).reshape(B, H, S, D).astype(np.float32)
    return full


# revision 29
# speedup vs baseline: 1.3823x; 1.3174x over previous
"""Causal multi-head attention (B=1, H=16, S=2048, D=128, fp32 I/O) on 8 trn2 cores.

Sharding: 2 heads per core (batch*head data parallel). Each core runs the same
Bass/Tile program on its own head pair.

Device algorithm (per head), v2:
  - Host supplies Q^T, K^T as fp16 [128 d, 2048 s] and V packed as fp16
    [128 k, 16*129] (per k-tile: 128 V columns + a ones column).
  - Stage 1 (per k-tile row kt): S^T[kt] = K_kt^T.T @ Q^T -> PSUM fp32 over the
    causal column range [kt*128, 2048), in 1-2 chunks. No mask matmuls: the
    strictly-future entries of the diagonal block are exp'd like everything
    else and then zeroed in SBUF by a GpSimd affine_select (GpSimd is idle).
  - exp is SPLIT between ScalarE (exact table exp, PSUM->SBUF fp16) and
    VectorE (Schraudolph fast exp: one tensor_scalar computing
    i16 = int(x*A + B) whose bit pattern IS fp16 exp(x*scale); |rel err| ~3%
    per element, ~0.4% on the softmax output because the denominator is
    computed from the same approximated values). Chunks are assigned to the
    two engines greedily by modeled cumulative cost, so both stay busy and
    the exp stream runs ~1.8x faster than ScalarE alone.
  - Stage 2 (per q-tile qt): accumulate over kt <= qt:
    acc[128 q, 129] += P^T[kt][:, qt-block].T @ V_aug[kt]
    -> columns 0..127 are the UNNORMALIZED output, column 128 the softmax
    denominator. Three consecutive q-tiles share one PSUM bank
    ([128, 3*129] <= 512 fp32), so one engine copy ships three tiles at once.
  - NO on-device normalization: the acc triples are copied PSUM->SBUF fp16
    (ScalarE/VectorE, balance-scheduled) and DMA'd out unnormalized; the host
    divides by the denominator column. This frees VectorE for exp work.

Startup: the two DMAs that gate the first matmul (K row 0, Q head) go on the
VECTOR DGE ring, whose framework preamble finishes ~1.5us before sync's.
Warm-up matmuls on a zeroed tile run during the DMA wait so the PE HAM clock
gate reaches 2.4 GHz before the real matmuls; the ScalarE exp table is
preloaded with a dummy activation at the same time.
"""

import os
import sys

import numpy as np

if "/opt/trn_rl_repo" not in sys.path:
    sys.path.insert(0, "/opt/trn_rl_repo")

B, H, S, D = 1, 16, 2048, 128
N_CORES = 8
HPC = H // N_CORES  # heads per core
NT = S // 128  # 16 seq tiles
VW = D + 1  # 129: V columns + ones column
SCALE = 1.0 / float(np.sqrt(D))
CHUNK = 1536  # stage-1 PSUM chunk (3 banks, 2 bufs -> depth-2 pipeline)

# Schraudolph fast-exp constants (fp16 bit space):
#   i16 = int(s_raw * EXP_A + EXP_B);  bitcast fp16 ~= exp(s_raw * SCALE)
# EXP_A folds the softmax scale; EXP_B = 15*1024 - c with c~60 minimizing the
# end-to-end softmax error (numpy-calibrated; insensitive to round-vs-trunc).
EXP_A = float(SCALE * 1024.0 * np.log2(np.e))
EXP_B = float(15 * 1024.0 - 60.0)

# Modeled per-instruction engine costs (ns) for the greedy exp/copy balance.
ACT_NS, ACT_FIX = 1.0 / 1.2, 257.0
DVE_NS, DVE_FIX = 1.0 / 0.96, 175.0

_CACHE = {}


def _build_program():
    if "nc" in _CACHE:
        return _CACHE["nc"]

    import concourse.bass as bass
    import concourse.mybir as mybir
    import concourse.tile as tile
    from concourse import bacc
    from contextlib import ExitStack

    f16 = mybir.dt.float16
    i16 = mybir.dt.int16
    f32 = mybir.dt.float32

    nc = bacc.Bacc("TRN2", target_bir_lowering=False, debug=False,
                   num_devices=N_CORES)

    qT = nc.dram_tensor("qT", [HPC, 128, S], f16, kind="ExternalInput").ap()
    kT = nc.dram_tensor("kT", [HPC, 128, S], f16, kind="ExternalInput").ap()
    vA = nc.dram_tensor("vA", [HPC, 128, NT * VW], f16, kind="ExternalInput").ap()
    # Output stays q-tile-partition-major ([q-offset, qt*VW+col]) so every
    # output DMA is per-partition contiguous; the host untangles it.
    out = nc.dram_tensor("out", [HPC, 128, NT * VW], f16,
                         kind="ExternalOutput").ap()

    with tile.TileContext(nc, pool_alloc_mode="queue") as tc, ExitStack() as ctx:
        const_pool = ctx.enter_context(tc.tile_pool(name="const", bufs=1))
        in_pool = ctx.enter_context(tc.tile_pool(name="qkv", bufs=2))
        # One buffer per exp-piece tile across both heads (~80 pieces), so
        # no stage-1 writer ever WAR-waits on a stage-2 reader.
        p_pool = ctx.enter_context(tc.tile_pool(name="pT", bufs=96))
        o_pool = ctx.enter_context(tc.tile_pool(name="osb", bufs=4))
        s_psum = ctx.enter_context(tc.tile_pool(name="spsum", bufs=6, space="PSUM"))
        a_psum = ctx.enter_context(tc.tile_pool(name="apsum", bufs=2, space="PSUM"))

        # PE warm-up: the HAM clock gate keeps TensorE at 1.2 GHz until it has
        # been busy ~3.4us. Run throwaway matmuls on a zeroed tile while the
        # first input DMAs are in flight; the real matmuls then extend the
        # busy streak so HAM reaches 2.4 GHz ~3.4us after the first warm-up.
        # The memset rides VectorE (idle, fast) so the warm-ups start the
        # moment the framework preamble barrier drops.
        warm_sb = const_pool.tile([128, 512], f16)
        nc.vector.memset(warm_sb[:], 0.0)
        warm_ps = s_psum.tile([128, 512], mybir.dt.float32, tag="s",
                              name="warm_ps")
        for _ in range(6):
            nc.tensor.matmul(warm_ps[:, 0:512], warm_sb[:, 0:128],
                             warm_sb[:, 0:512], start=True, stop=True)
        # Preload the ScalarE exp table set during the DMA wait (walrus puts
        # the ACT_TABLE_LOAD right before this first ACTIVATE).
        warm_exp = const_pool.tile([128, 1], f16)
        nc.scalar.activation(warm_exp[:], warm_sb[:, 0:1],
                             mybir.ActivationFunctionType.Exp, scale=SCALE)

        qk_sb = {}   # h -> (qT_sb, kT_sb, vA_sb)
        pT = {}      # h -> list of P^T row tiles

        def emit_loads(h, first=False):
            qT_sb = in_pool.tile([128, S], f16, tag="q", name=f"q_{h}")
            kT_sb = in_pool.tile([128, S], f16, tag="k", name=f"k_{h}")
            vA_sb = in_pool.tile([128, NT * VW], f16, tag="v", name=f"v_{h}")
            if first:
                # Rows 12-15 run FIRST (see rows_order): they need only the
                # tail 640 columns of K and Q (320 KB), which land ~2us
                # before the full Q does at the ~180 GB/s per-queue DMA rate.
                # Those rows bridge the PE from the warm-up matmuls to row 0
                # with no idle gap, so the HAM clock gate reaches 2.4 GHz
                # before the bulk of the work. Q's head is split across the
                # sync and gpsimd rings so both queues carry it in parallel.
                nc.sync.dma_start(kT_sb[:, 1408:2048], kT[h][:, 1408:2048])
                nc.sync.dma_start(qT_sb[:, 1408:2048], qT[h][:, 1408:2048])
                nc.sync.dma_start(qT_sb[:, 0:704], qT[h][:, 0:704])
                nc.sync.dma_start(kT_sb[:, 0:128], kT[h][:, 0:128])
                nc.sync.dma_start(kT_sb[:, 128:640], kT[h][:, 128:640])
                nc.sync.dma_start(vA_sb[:, 0:6 * VW], vA[h][:, 0:6 * VW])
                nc.gpsimd.dma_start(qT_sb[:, 704:1408], qT[h][:, 704:1408])
                nc.gpsimd.dma_start(kT_sb[:, 640:1408], kT[h][:, 640:1408])
                nc.gpsimd.dma_start(vA_sb[:, 6 * VW:], vA[h][:, 6 * VW:])
            else:
                # Later heads run in natural row order: K row 0 + full Q
                # first, K tail and V after (first-use order).
                nc.sync.dma_start(kT_sb[:, 0:128], kT[h][:, 0:128])
                nc.sync.dma_start(qT_sb[:, 0:2048], qT[h][:, 0:2048])
                nc.sync.dma_start(kT_sb[:, 128:2048], kT[h][:, 128:2048])
                nc.sync.dma_start(vA_sb[:], vA[h])
            qk_sb[h] = (qT_sb, kT_sb, vA_sb)
            pT[h] = {kt: [] for kt in range(NT)}  # kt -> [(g_lo, g_hi, tile)]

        # Greedy ACT/DVE balance for exp chunks and acc copies.
        eng_t = {"act": 0.0, "dve": 0.0}

        def pick_engine():
            return "act" if eng_t["act"] <= eng_t["dve"] else "dve"

        def emit_exp(engine, h, kt, lo, hi, sp, sp_lo):
            # exp of score chunk cols [lo, hi) of row kt (global q coords),
            # reading PSUM tile sp at offset lo - sp_lo. Each piece gets its
            # OWN SBUF tile: the Tile framework serializes two engines that
            # write disjoint ranges of one tile (cross-engine same-tile
            # hazard), which would turn the ACT||DVE split back into a
            # serial stream. One writer per tile keeps them parallel.
            n = hi - lo
            t = p_pool.tile([128, n], f16, tag="p", name=f"p_{h}_{kt}_{lo}")
            pT[h][kt].append((lo, hi, t))
            src = sp[:, lo - sp_lo:hi - sp_lo]
            if engine == "act":
                nc.scalar.activation(t[:], src,
                                     mybir.ActivationFunctionType.Exp,
                                     scale=SCALE)
                eng_t["act"] += n * ACT_NS + ACT_FIX
            else:
                nc.vector.tensor_scalar(
                    t[:].bitcast(i16), src, EXP_A, EXP_B,
                    mybir.AluOpType.mult, mybir.AluOpType.add)
                eng_t["dve"] += n * DVE_NS + DVE_FIX

        def stage1(h, kt):
            # The row is processed in 512-col pieces, each with its OWN
            # one-bank PSUM tile and its own exp instruction on ONE engine.
            # The Tile scheduler CHAINS cross-engine readers of a shared
            # tile (its event-accel workaround), so two engines exp'ing
            # halves of one PSUM chunk run serially — per-piece tiles keep
            # ScalarE and VectorE genuinely parallel, and 6 one-bank bufs
            # give a depth-6 matmul->exp pipeline.
            qT_sb, kT_sb, _ = qk_sb[h]
            c0 = kt * 128
            k_blk = kT_sb[:, c0:c0 + 128]
            L = S - c0
            cc = c0
            first = True
            while cc < c0 + L:
                plen = min(512, c0 + L - cc)
                sp = s_psum.tile([128, 512], mybir.dt.float32, tag="s",
                                 name=f"sp_{h}_{kt}_{cc}")
                nc.tensor.matmul(sp[:, 0:plen], k_blk,
                                 qT_sb[:, cc:cc + plen],
                                 start=True, stop=True)
                emit_exp(pick_engine(), h, kt, cc, cc + plen, sp, cc)
                if first:
                    # Zero the strictly-future entries of the diagonal block
                    # (k > q <=> partition p > col j) now that exp ran. The
                    # subsequent PV matmuls and the ones-column denominator
                    # then see exact causal zeros. GpSimd is otherwise idle.
                    diag = pT[h][kt][0][2][:, 0:128]
                    nc.gpsimd.affine_select(
                        diag, diag, pattern=[[1, 128]],
                        compare_op=mybir.AluOpType.is_ge, fill=0.0,
                        base=0, channel_multiplier=-1)
                cc += plen
                first = False

        accs = {}

        def ship_triple(h, trip, nq):
            # Copy the finished acc triple PSUM->SBUF fp16 on the engine
            # that's ahead, then DMA it out unnormalized (host divides).
            acc = accs[(h, trip)]
            w = nq * VW
            osb = o_pool.tile([128, w], f16, tag="o", name=f"osb_{h}_{trip}")
            eng = pick_engine()
            if eng == "act":
                nc.scalar.copy(osb[:], acc[:, :w])
                eng_t["act"] += w * ACT_NS + ACT_FIX
            else:
                nc.vector.tensor_copy(osb[:], acc[:, :w])
                eng_t["dve"] += w * DVE_NS + DVE_FIX
            dst = out[h][:, trip * 3 * VW:(trip * 3 + nq) * VW]
            # Output DMAs ride the idle gpsimd ring; the tail triples of the
            # last head go on sync, which is free once inputs are done.
            q = nc.sync if (h == HPC - 1 and trip >= 4) else nc.gpsimd
            q.dma_start(dst, osb[:])

        def stage2_piece(h, qt, lo, hi):
            # One slice of the PV accumulation group for q-tile qt. PSUM
            # accumulation is per-element, so the group's matmuls need not be
            # contiguous on the PE stream.
            vA_sb = qk_sb[h][2]
            q0 = qt * 128
            trip, slot = qt // 3, qt % 3
            if lo == 0 and slot == 0:
                accs[(h, trip)] = a_psum.tile([128, 3 * VW], mybir.dt.float32,
                                              tag="acc", name=f"acc_{h}_{trip}")
            acc = accs[(h, trip)][:, slot * VW:(slot + 1) * VW]
            for k2 in range(lo, hi):
                for (g_lo, g_hi, t) in pT[h][k2]:
                    if g_lo <= q0 < g_hi:
                        blk = t[:, q0 - g_lo:q0 - g_lo + 128]
                        break
                else:
                    raise AssertionError((h, k2, q0))
                nc.tensor.matmul(
                    acc,
                    blk,
                    vA_sb[:, k2 * VW:(k2 + 1) * VW],
                    start=(k2 == 0), stop=(k2 == qt),
                )
            if hi == qt + 1 and (slot == 2 or qt == NT - 1):
                ship_triple(h, trip, slot + 1)

        # One flat software pipeline across both heads. Each head's rows run
        # in the order [12..15, 0..11]: the tiny tail rows bridge the PE over
        # the Q-head DMA window (they need only late K/Q columns). Stage-2
        # pieces are slotted by data-readiness: a piece runs 1-2 slots after
        # the LAST stage-1 row it depends on. Groups 10-15 therefore drain at
        # the head boundary, naturally interleaving with the next head's
        # early rows (PE chews PV bulks while the exp engines chew the next
        # head's scores).
        # Head 0 needs the bridge reorder; later heads' inputs land while
        # head 0 computes, so they run in natural order — their PV groups
        # then spread across their own rows and the final drain is short.
        PRE = [12, 13, 14, 15]
        seq = []
        for h in range(HPC):
            order = PRE + list(range(12)) if h == 0 else list(range(NT))
            seq += [(h, kt) for kt in order]
        gpos = {hk: i for i, hk in enumerate(seq)}
        NSLOT = len(seq) + 6
        pieces = [[] for _ in range(NSLOT)]

        # Every group is ONE whole piece, slotted 2 after its last required
        # row. Splitting groups into early-bulk pieces looks tempting but
        # overlaps three acc-triple lifetimes, which WAR-deadlocks the
        # in-order PE against the 2-buf a_psum pool. Whole groups keep the
        # triple windows strictly 2-deep: triple t ships at slot 3t+8,
        # before triple t+2 allocates at slot 3t+12.
        for h in range(HPC):
            for qt in range(NT):
                ready = max(gpos[(h, r)] for r in range(qt + 1))
                pieces[min(NSLOT - 1, ready + 2)].append((h, qt, 0, qt + 1))

        emit_loads(0, first=True)
        started = {0}
        for i, (h, kt) in enumerate(seq):
            if h + 1 < HPC and kt == 15 and (h + 1) not in started:
                emit_loads(h + 1)
                started.add(h + 1)
            stage1(h, kt)
            for p in pieces[i]:
                stage2_piece(*p)
        for pl in pieces[len(seq):]:
            for p in pl:
                stage2_piece(*p)

    nc.compile()
    _CACHE["nc"] = nc
    return nc


def _host_prep(query_states, key_states, value_states):
    """Per-core input maps: fp16 Q^T/K^T and ones-augmented V."""
    q = np.asarray(query_states, dtype=np.float32).reshape(H, S, D)
    k = np.asarray(key_states, dtype=np.float32).reshape(H, S, D)
    v = np.asarray(value_states, dtype=np.float32).reshape(H, S, D)

    in_maps = []
    for c in range(N_CORES):
        hs = slice(c * HPC, (c + 1) * HPC)
        qT = np.ascontiguousarray(
            q[hs].transpose(0, 2, 1).astype(np.float16))  # [HPC,128,S]
        kT = np.ascontiguousarray(
            k[hs].transpose(0, 2, 1).astype(np.float16))
        vh = v[hs].astype(np.float16).reshape(HPC, NT, 128, D)
        vA = np.empty((HPC, 128, NT * VW), dtype=np.float16)
        for hh in range(HPC):
            for kt in range(NT):
                vA[hh, :, kt * VW:kt * VW + D] = vh[hh, kt]
                vA[hh, :, kt * VW + D] = np.float16(1.0)
        in_maps.append({"qT": qT, "kT": kT, "vA": vA})
    return in_maps


def run_cores(in_maps, trace=False, **kw):
    from concourse.bass_utils import run_bass_kernel_spmd
    nc = _build_program()
    return run_bass_kernel_spmd(nc, in_maps, list(range(N_CORES)),
                                trace=trace, **kw)


def kernel(query_states, key_states, value_states, attention_mask=None,
           attention_dropout=None, **_ignored):
    in_maps = _host_prep(query_states, key_states, value_states)
    res = run_cores(in_maps)
    outs = []
    for c in range(N_CORES):
        o = np.asarray(res.results[c]["out"], dtype=np.float32)  # [HPC,128,NT*VW]
        o = o.reshape(HPC, 128, NT, VW).transpose(0, 2, 1, 3)  # [HPC,NT,128,VW]
        o = o[..., :D] / o[..., D:D + 1]  # host-side softmax normalization
        outs.append(o.reshape(HPC, S, D))
    full = np.concatenate(outs, axis=0).reshape(B, H, S, D).astype(np.float32)
    return full
